# revision 1
# baseline (speedup 1.0000x reference)
"""AttentionTagClassifier Trainium2 kernel.

Data-parallel over batch: 32 sequences -> 4 per NeuronCore, weights
replicated.  Two bass programs per core (encoder+precomputes, decoder).

Math (per core, BL=4), validated against reference in numpy first:
 - encoder x-gates as one GEMM with bias folded via an appended ones-row
 - bidirectional LSTM recurrence: stationary = h^T chunks (128,4), the
   weight matrix streams through the PE as the moving operand
 - decoder reformulation:
     * E_proj = tag_embed @ W_emb.T precomputed (host, weight-only), so the
       tag-embedding contribution to the gates is a row gather
     * X_al[t] = aligned_t @ W_al.T + biases precomputed on-chip (GEMM)
     * context never materialized: [W_ctx; Wout_c] @ context ==
       (W_comb @ output_b^T) @ probs_b, with OP_comb precomputed per b and
       the per-step contraction done as a 4-batch block-diagonal matmul
       over T-subchunks straight into the gates/scores PSUM
     * attention scores sc_b = P_b @ h_b with P = output @ attn_W.T
       precomputed; per-b M=1 matmuls with h^T columns as stationary
 - gate order is host-permuted [i f o g] so one sigmoid covers i,f,o
 - argmax over V=2048 via reshape to (128,64) + max_with_indices, then a
   PE transpose and a masked min-reduce for the global (first-max) index
"""
import sys, os
sys.path.insert(0, "/opt/trn_rl_repo")
import numpy as np

import bass_rust
import concourse.bass as bass
import concourse.mybir as mybir
import concourse.tile as tile_mod
from concourse.bass import IndirectOffsetOnAxis
from concourse.bass_utils import run_bass_kernel_spmd
from concourse.bacc import Bacc

F32 = mybir.dt.float32
BF16 = mybir.dt.bfloat16
U32 = mybir.dt.uint32
I32 = mybir.dt.int32
AF = mybir.ActivationFunctionType
ALU = mybir.AluOpType
AX = mybir.AxisListType

B, T, D, H, E, V = 32, 64, 512, 512, 512, 2048
H2 = 2 * H           # 1024
GE = 4 * H           # 2048 encoder gates
GD = 4 * H2          # 4096 decoder gates
NC_N = 8
BL = B // NC_N       # 4
PCOMB = GD + V       # 6144 = [W_ctx(4096); Wout_c(2048)] projection rows


# ---------------------------------------------------------------- tile patch
def _patched_drain_and_barrier(self, tick_clock, wait_clock):
    """This walrus build rejects >1 sync wait per instruction; the Tile tail
    piles every processor's wait onto one Drain.  Split: one Drain each."""
    nc = self.nc
    drain_inst = nc.sync.drain()
    wait_clock.add_sem_waits(
        drain_inst.ins, tile_mod.ScopedClock({None: tick_clock.global_clock})
    )
    si = drain_inst.ins.sync_info
    waits = list(si.on_wait) if si is not None else []
    if len(waits) > 1:
        drain_inst.ins.sync_info = bass_rust.SyncInfo(
            on_wait=[waits[0]], on_update=list(si.on_update)
        )
        for w in waits[1:]:
            d2 = nc.sync.drain()
            d2.ins.sync_info = bass_rust.SyncInfo(on_wait=[w], on_update=[])
    nc.all_engine_barrier()
    assert self.sems is not None
    popped = nc._tile_sem_poison_stack.pop()
    assert popped is self._sem_poison
    nc.clear_and_free_semaphores(list(self.sems.allocated().values()))
    nc.all_engine_barrier()


tile_mod.TileContext._drain_and_barrier = _patched_drain_and_barrier


# ---------------------------------------------------------------- host prep
def host_prep(w):
    p = {}
    for d_ in ("f", "b"):
        wih = np.asarray(w[f"enc_Wih_{d_}"], np.float32)
        whh = np.asarray(w[f"enc_Whh_{d_}"], np.float32)
        bias = np.asarray(w[f"enc_bih_{d_}"], np.float32) + np.asarray(
            w[f"enc_bhh_{d_}"], np.float32)
        # reorder encoder gates [i f g o] -> [i f o g]
        perm = np.concatenate([np.arange(0, H), np.arange(H, 2 * H),
                               np.arange(3 * H, 4 * H), np.arange(2 * H, 3 * H)])
        wih, whh, bias = wih[perm], whh[perm], bias[perm]
        p[f"wihT_aug_{d_}"] = np.ascontiguousarray(
            np.concatenate([wih.T, bias[None, :]], axis=0))       # (513, 2048)
        p[f"whhT_{d_}"] = np.ascontiguousarray(whh.T)             # (512, 2048)

    permd = np.concatenate([np.arange(0, H2), np.arange(H2, 2 * H2),
                            np.arange(3 * H2, 4 * H2), np.arange(2 * H2, 3 * H2)])
    dec_Wih = np.asarray(w["dec_Wih"], np.float32)[permd]
    dec_Whh = np.asarray(w["dec_Whh"], np.float32)[permd]
    dec_bias = (np.asarray(w["dec_bih"], np.float32)
                + np.asarray(w["dec_bhh"], np.float32))[permd]
    W_emb = dec_Wih[:, :E]
    W_ctx = dec_Wih[:, E:E + H2]
    W_al = dec_Wih[:, E + H2:]
    p["whhT_dec"] = np.ascontiguousarray(dec_Whh.T)               # (1024, 4096)
    p["e_proj"] = np.ascontiguousarray(
        np.asarray(w["tag_embed"], np.float32) @ W_emb.T)         # (2048, 4096)
    p["walT_aug"] = np.ascontiguousarray(
        np.concatenate([W_al.T, dec_bias[None, :]], axis=0))      # (1025, 4096)
    Wout = np.asarray(w["Wout"], np.float32)
    p["wout_hT"] = np.ascontiguousarray(Wout[:, :H2].T)           # (1024, 2048)
    p["w_combT"] = np.ascontiguousarray(
        np.concatenate([W_ctx, Wout[:, H2:]], axis=0).T)          # (1024, 6144)
    p["attn_WT"] = np.ascontiguousarray(
        np.asarray(w["attn_W"], np.float32).T)                    # (1024, 1024)
    p["bout_row"] = np.asarray(w["bout"], np.float32)[None, :]    # (1, 2048)
    p["ident"] = np.eye(128, dtype=np.float32)
    p["i8"] = np.concatenate([np.eye(4, dtype=np.float32)] * 2, axis=0)  # (8,4)
    return p


# ------------------------------------------------------------- enc program
def build_enc():
    nc = bass.Bass()
    dp = lambda n, s, dt=F32, out=False: nc.declare_dram_parameter(
        n, list(s), dt, isOutput=out)
    embT = dp("embT_aug", (513, BL * T))
    wihT = {d: dp(f"wihT_aug_{d}", (513, GE)) for d in "fb"}
    whhT = {d: dp(f"whhT_{d}", (H, GE)) for d in "fb"}
    attn_WT = dp("attn_WT", (H2, H2))
    w_combT = dp("w_combT", (H2, PCOMB))
    walT = dp("walT_aug", (H2 + 1, GD))
    boutd = dp("bout_row", (1, V))
    i8d = dp("i8", (8, 4))
    identd = dp("ident", (128, 128))

    xal = dp("xal", (BL, T, GD), out=True)
    pti = dp("pti", (8, 128, BL * T), out=True)
    op4t = dp("op4t", (2, 128, PCOMB), out=True)
    lastT_d = dp("lastT", (128, 32), out=True)

    xg = {d: nc.dram_tensor(f"xg_{d}", [BL, T, GE], F32) for d in "fb"}

    with tile_mod.TileContext(nc) as tc:
        with (
            tc.tile_pool(name="res", bufs=1) as res,       # resident, unique tags
            tc.tile_pool(name="wpool", bufs=1) as wpool,  # big weight tiles
            tc.tile_pool(name="stream", bufs=4) as stream,
            tc.tile_pool(name="work", bufs=3) as work,
            tc.tile_pool(name="pgp", bufs=1, space="PSUM") as pgp,
            tc.tile_pool(name="ptr", bufs=2, space="PSUM") as ptr,
            tc.tile_pool(name="pp3", bufs=2, space="PSUM") as pp3,
        ):
            ident = res.tile([128, 128], F32, tag="ident")
            nc.sync.dma_start(ident[:], identd[:])
            i8 = res.tile([8, 4], F32, tag="i8")
            nc.sync.dma_start(i8[:], i8d[:])
            ones_row = res.tile([1, BL * T], F32, tag="ones")
            nc.vector.memset(ones_row[:], 1.0)
            boutr = res.tile([1, V], F32, tag="boutr")
            nc.sync.dma_start(boutr[:], boutd[:])

            # ---------------- phase 1: x-gates GEMMs
            et = [res.tile([128, BL * T], F32, tag=f"otif{k}", name=f"et{k}") for k in range(4)]
            for k in range(4):
                nc.sync.dma_start(et[k][:], embT[128 * k:128 * (k + 1), :])
            et4 = res.tile([1, BL * T], F32, tag="et4")
            nc.sync.dma_start(et4[:], embT[512:513, :])

            for d_ in "fb":
                wi = [wpool.tile([128, GE], F32, tag=f"bw{k}", name=f"wi{k}") for k in range(4)]
                for k in range(4):
                    nc.sync.dma_start(wi[k][:], wihT[d_][128 * k:128 * (k + 1), :])
                wib = work.tile([1, GE], F32, tag="wib", bufs=1)
                nc.sync.dma_start(wib[:], wihT[d_][512:513, :])
                xgv = xg[d_][:].rearrange("b t g -> (b t) g")
                for m in range(2):
                    for n in range(4):
                        ps = pp3.tile([128, 512], F32, tag="p3")
                        for k in range(4):
                            nc.tensor.matmul(
                                ps[:], et[k][:, 128 * m:128 * (m + 1)],
                                wi[k][:, 512 * n:512 * (n + 1)],
                                start=(k == 0), stop=False)
                        nc.tensor.matmul(
                            ps[:], et4[:, 128 * m:128 * (m + 1)],
                            wib[:, 512 * n:512 * (n + 1)],
                            start=False, stop=True)
                        sb = work.tile([128, 512], F32, tag="xgout")
                        nc.scalar.activation(sb[:], ps[:], AF.Copy)
                        nc.sync.dma_start(
                            xgv[128 * m:128 * (m + 1), 512 * n:512 * (n + 1)],
                            sb[:])

            # ---------------- phase 2: recurrence
            wr = {d: [wpool.tile([128, GE], F32, tag=f"bw{(0 if d == chr(102) else 4) + k}", name=f"whh{d}{k}")
                      for k in range(4)] for d in "fb"}
            for d_ in "fb":
                for k in range(4):
                    nc.sync.dma_start(
                        wr[d_][k][:], whhT[d_][128 * k:128 * (k + 1), :])
            outTI = {d: [res.tile([128, BL * T], F32, tag=f"otix{d}{k}", name=f"oti{d}{k}")
                         for k in range(4)] for d in "fb"}
            cst = {d: res.tile([BL, H], F32, tag=f"c{d}", name=f"c{d}") for d in "fb"}
            for d_ in "fb":
                nc.vector.memset(cst[d_][:], 0.0)
            lastT = res.tile([128, 32], F32, tag="lastT")

            for step in range(T):
                for d_, t in (("f", step), ("b", T - 1 - step)):
                    tprev = t - 1 if d_ == "f" else t + 1
                    xgt = stream.tile([BL, GE], F32, tag="xgt", bufs=2)
                    nc.sync.dma_start(xgt[:], xg[d_][:, t, :])
                    gp = pgp.tile([BL, GE], F32, tag="gp")
                    for n in range(4):
                        if step > 0:
                            for k in range(4):
                                hT_sl = outTI[d_][k][:].rearrange(
                                    "p (b t) -> p b t", b=BL)[:, :, tprev]
                                nc.tensor.matmul(
                                    gp[:, 512 * n:512 * (n + 1)], hT_sl,
                                    wr[d_][k][:, 512 * n:512 * (n + 1)],
                                    start=(k == 0), stop=False)
                        nc.tensor.matmul(
                            gp[:, 512 * n:512 * (n + 1)], i8[0:4, :],
                            xgt[:, 512 * n:512 * (n + 1)],
                            start=(step == 0), stop=True)
                    # cell ([i f o g] order)
                    sfo = work.tile([BL, 3 * H], F32, tag="sfo", bufs=1)
                    nc.scalar.activation(sfo[:], gp[:, 0:3 * H], AF.Sigmoid)
                    tg = work.tile([BL, H], F32, tag="tg", bufs=1)
                    nc.scalar.activation(tg[:], gp[:, 3 * H:4 * H], AF.Tanh)
                    t1 = work.tile([BL, H], F32, tag="t1", bufs=1)
                    nc.vector.tensor_mul(t1[:], sfo[:, 0:H], tg[:])
                    t2 = work.tile([BL, H], F32, tag="t2", bufs=1)
                    nc.vector.tensor_mul(t2[:], sfo[:, H:2 * H], cst[d_][:])
                    nc.vector.tensor_add(cst[d_][:], t1[:], t2[:])
                    tcel = work.tile([BL, H], F32, tag="tcel", bufs=1)
                    nc.scalar.activation(tcel[:], cst[d_][:], AF.Tanh)
                    htile = work.tile([BL, H], F32, tag="h", bufs=1)
                    nc.vector.tensor_mul(htile[:], sfo[:, 2 * H:3 * H], tcel[:])
                    for k in range(4):
                        tp = ptr.tile([128, BL], F32, tag="tr")
                        nc.tensor.transpose(
                            tp[:], htile[:, 128 * k:128 * (k + 1)],
                            ident[0:4, 0:4])
                        dst = outTI[d_][k][:].rearrange(
                            "p (b t) -> p b t", b=BL)[:, :, t]
                        nc.vector.tensor_copy(dst, tp[:])
                    if (d_ == "f" and step == T - 1) or (d_ == "b" and step == 0):
                        off = 0 if d_ == "f" else 16
                        for k in range(4):
                            src = outTI[d_][k][:].rearrange(
                                "p (b t) -> p b t", b=BL)[:, :, T - 1]
                            nc.vector.tensor_copy(
                                lastT[:, off + 4 * k:off + 4 * k + 4], src)
            nc.sync.dma_start(lastT_d[:], lastT[:])

            oti8 = outTI["f"] + outTI["b"]  # d-chunks 0..7 of [out_f; out_b]

            # ---------------- phase 3a: X_al
            xalv = xal[:].rearrange("b t g -> (b t) g")
            wbias = res.tile([1, GD], F32, tag="walbias")
            nc.sync.dma_start(wbias[:], walT[H2:H2 + 1, :])
            for m in range(2):
                for n in range(8):
                    ps = pp3.tile([128, 512], F32, tag="p3", name="p3a")
                    for k in range(8):
                        wals = stream.tile([128, 512], F32, tag="wals",
                                           name="wals", bufs=3)
                        nc.sync.dma_start(
                            wals[:], walT[128 * k:128 * (k + 1),
                                          512 * n:512 * (n + 1)])
                        nc.tensor.matmul(
                            ps[:], oti8[k][:, 128 * m:128 * (m + 1)],
                            wals[:], start=(k == 0), stop=False)
                    nc.tensor.matmul(
                        ps[:], ones_row[:, 128 * m:128 * (m + 1)],
                        wbias[:, 512 * n:512 * (n + 1)], start=False, stop=True)
                    sb = work.tile([128, 512], F32, tag="xalout")
                    nc.scalar.activation(sb[:], ps[:], AF.Copy)
                    nc.sync.dma_start(
                        xalv[128 * m:128 * (m + 1), 512 * n:512 * (n + 1)],
                        sb[:])

            # ---------------- phase 3b: PTI  (P^T chunks)
            for e in range(8):
                pse = pp3.tile([128, BL * T], F32, tag="p3")
                for k in range(8):
                    awt = stream.tile([128, 128], F32, tag="awt")
                    nc.sync.dma_start(
                        awt[:], attn_WT[128 * k:128 * (k + 1),
                                        128 * e:128 * (e + 1)])
                    nc.tensor.matmul(pse[:], awt[:], oti8[k][:],
                                     start=(k == 0), stop=(k == 7))
                sb = work.tile([128, BL * T], F32, tag="ptiout")
                nc.scalar.activation(sb[:], pse[:], AF.Copy)
                nc.sync.dma_start(pti[e], sb[:])

            # ---------------- phase 3c: OP4T round tiles (bf16)
            for n in range(PCOMB // 512):
                ppa = pp3.tile([128, 512], F32, tag="p3")
                ppb = pp3.tile([128, 512], F32, tag="p3")
                is_v = n >= GD // 512   # cols carrying Wout_c rows: add bout
                for k in range(8):
                    wcb = stream.tile([128, 512], F32, tag="wcb")
                    nc.sync.dma_start(
                        wcb[:], w_combT[128 * k:128 * (k + 1),
                                        512 * n:512 * (n + 1)])
                    for b in range(4):
                        pp = ppa if b < 2 else ppb
                        ro = 64 * (b % 2)
                        nc.tensor.matmul(
                            pp[ro:ro + 64, :], oti8[k][:, 64 * b:64 * (b + 1)],
                            wcb[:], start=(k == 0),
                            stop=(k == 7 and not is_v))
                if is_v:
                    bsl = boutr[:, 512 * n - GD:512 * (n + 1) - GD]
                    for pp in (ppa, ppb):
                        for ro in (0, 64):
                            nc.tensor.matmul(
                                pp[ro:ro + 64, :], ones_row[:, 0:64], bsl,
                                start=False, stop=True)
                for b in range(4):
                    pp = ppa if b < 2 else ppb
                    ro = 64 * (b % 2)
                    for r in range(2):
                        opc = work.tile([32, 512], F32, tag="op4c", name="op4c")
                        nc.vector.tensor_copy(
                            opc[:], pp[ro + 32 * r:ro + 32 * (r + 1), :])
                        nc.sync.dma_start(
                            op4t[r][32 * b:32 * (b + 1), 512 * n:512 * (n + 1)],
                            opc[:])
    bass_rust.generate_event_semaphores(nc)
    return nc


# ------------------------------------------------------------- dec program
def build_dec():
    nc = bass.Bass()
    dp = lambda n, s, dt=F32, out=False: nc.declare_dram_parameter(
        n, list(s), dt, isOutput=out)
    whhd = dp("whhT_dec", (H2, GD))
    woutd = dp("wout_hT", (H2, V))
    eproj = dp("e_proj", (V, GD))
    xald = dp("xal", (BL, T, GD))
    ptid = dp("pti", (8, 128, BL * T))
    op4d = dp("op4t", (2, 128, PCOMB))
    lastd = dp("lastT", (128, 32))
    i8d = dp("i8", (8, 4))
    identd = dp("ident", (128, 128))
    scores = dp("scores", (BL, T, V), out=True)
    DBG = bool(int(__import__("os").environ.get("KDBG", "0")))
    if DBG:
        dbg_embx = dp("dbg_embx", (3, 8, GD), out=True)
        dbg_ht = dp("dbg_ht", (3, 128, 32), out=True)
        dbg_sfo = dp("dbg_sfo", (3, BL, 2 * H2), out=True)
        dbg_shuf = dp("dbg_shuf", (128, 64), out=True)
        dbg_rowv = dp("dbg_rowv", (2, 128), out=True)
        dbg_gl = dp("dbg_gl", (1, 128), out=True)
        dbg_tagsf = dp("dbg_tagsf", (BL, 1), out=True)
        dbg_mwi = dp("dbg_mwi", (128, 16), out=True)

    with tile_mod.TileContext(nc) as tc:
        with (
            tc.tile_pool(name="res", bufs=1) as res,
            tc.tile_pool(name="stream", bufs=2) as stream,
            tc.tile_pool(name="work", bufs=1) as work,
            tc.tile_pool(name="pbig", bufs=1, space="PSUM") as pbig,
            tc.tile_pool(name="ptr", bufs=1, space="PSUM") as ptr,
            tc.tile_pool(name="psc", bufs=2, space="PSUM") as psc,
            tc.tile_pool(name="ppt", bufs=1, space="PSUM") as ppt,
        ):
            ident = res.tile([128, 128], F32, tag="ident")
            nc.sync.dma_start(ident[:], identd[:])
            i8 = res.tile([8, 4], F32, tag="i8")
            nc.sync.dma_start(i8[:], i8d[:])
            w = [res.tile([128, GD], F32, tag=f"w{k}", name=f"w{k}") for k in range(8)]
            for k in range(8):
                nc.sync.dma_start(w[k][:], whhd[128 * k:128 * (k + 1), :])
            p = [res.tile([128, BL * T], F32, tag=f"p{k}", name=f"p{k}") for k in range(8)]
            for k in range(8):
                nc.sync.dma_start(p[k][:], ptid[k])
            lastT = res.tile([128, 32], F32, tag="lastT")
            nc.sync.dma_start(lastT[:], lastd[:])
            probs4 = [res.tile([128, 4], F32, tag=f"pr4{r}", name=f"pr4{r}") for r in range(2)]
            for r in range(2):
                nc.vector.memset(probs4[r][:], 0.0)
            hT = res.tile([128, 32], F32, tag="hT")
            cst = res.tile([BL, H2], F32, tag="c")
            nc.vector.memset(cst[:], 0.0)
            embX = res.tile([8, GD], F32, tag="embX")
            nc.vector.memset(embX[0:4, :], 0.0)
            nc.sync.dma_start(embX[4:8, :], xald[:, 0, :])
            giti = res.tile([1, 128], I32, tag="giti")
            nc.gpsimd.iota(
                giti[:].rearrange("a (x y) -> a x y", x=4),
                pattern=[[0, 4], [64, 32]], base=0, channel_multiplier=0)
            gidxf = res.tile([1, 128], F32, tag="gidxf")
            nc.vector.tensor_copy(gidxf[:], giti[:])

            def attention(hT_src):
                """sc_b = P_b @ h_b -> softmax -> refill probs4 (for next step).
                All compute stays on partition 0 per b (32-align rule)."""
                for b in range(4):
                    scp = psc.tile([1, T], F32, tag="sc", name="scp")
                    for k in range(8):
                        nc.tensor.matmul(
                            scp[:], hT_src[:, 4 * k + b:4 * k + b + 1],
                            p[k][:, T * b:T * (b + 1)],
                            start=(k == 0), stop=(k == 7))
                    esc = work.tile([1, T], F32, tag="esc", bufs=2)
                    ssum = work.tile([1, 1], F32, tag="ssum", bufs=2)
                    nc.scalar.activation(
                        esc[:], scp[:], AF.Exp, accum_out=ssum[:])
                    rs = work.tile([1, 1], F32, tag="rs", bufs=2)
                    nc.vector.reciprocal(rs[:], ssum[:])
                    pr = work.tile([1, T], F32, tag="pr", bufs=2)
                    nc.vector.tensor_scalar_mul(pr[:], esc[:], rs[:])
                    pT = ppt.tile([T, 1], F32, tag="pt", name="pTb")
                    nc.tensor.transpose(pT[:], pr[:], ident[0:1, 0:1])
                    for r in range(2):
                        nc.vector.tensor_copy(
                            probs4[r][32 * b:32 * (b + 1), b:b + 1],
                            pT[32 * r:32 * (r + 1), :])

            attention(lastT)

            for t in range(T):
                if DBG and t < 3:
                    nc.sync.dma_start(dbg_embx[t], embX[:])
                # ---- gates (two halves of 2048) into PSUM
                sfo_if = work.tile([BL, 2 * H2], F32, tag="big1")
                sfo_o = work.tile([BL, H2], F32, tag="sfoo")
                tg = work.tile([BL, H2], F32, tag="tg")
                for half in range(2):
                    gp = pbig.tile([BL, 2 * H2], F32, tag="big", name="gp")
                    for n in range(4):
                        col = 2048 * half + 512 * n
                        started = False
                        if t > 0:
                            for k in range(8):
                                nc.tensor.matmul(
                                    gp[:, 512 * n:512 * (n + 1)],
                                    hT[:, 4 * k:4 * k + 4],
                                    w[k][:, col:col + 512],
                                    start=(k == 0), stop=False)
                            started = True
                        for r in range(2):
                            opst = stream.tile([128, 512], F32, tag="opst",
                                               name="opst", bufs=3)
                            nc.sync.dma_start(
                                opst[:], op4d[r][:, col:col + 512])
                            nc.tensor.matmul(
                                gp[:, 512 * n:512 * (n + 1)], probs4[r][:],
                                opst[:],
                                start=(not started and r == 0), stop=False)
                        nc.tensor.matmul(
                            gp[:, 512 * n:512 * (n + 1)], i8[:],
                            embX[:, col:col + 512], start=False, stop=True)
                    if half == 0:
                        nc.scalar.activation(sfo_if[:], gp[:], AF.Sigmoid)
                        if DBG and t < 3:
                            nc.sync.dma_start(dbg_sfo[t], sfo_if[:])
                    else:
                        nc.scalar.activation(sfo_o[:], gp[:, 0:H2], AF.Sigmoid)
                        nc.scalar.activation(tg[:], gp[:, H2:2 * H2], AF.Tanh)
                # ---- cell (in-place: c = f*c + i*tanh(g); h = o*tanh(c))
                nc.vector.tensor_mul(cst[:], cst[:], sfo_if[:, H2:2 * H2])
                nc.vector.tensor_mul(tg[:], tg[:], sfo_if[:, 0:H2])
                nc.vector.tensor_add(cst[:], cst[:], tg[:])
                nc.scalar.activation(tg[:], cst[:], AF.Tanh)
                htile = work.tile([BL, H2], F32, tag="h")
                nc.vector.tensor_mul(htile[:], sfo_o[:], tg[:])
                for k in range(8):
                    tp = ptr.tile([128, BL], F32, tag="tr")
                    nc.tensor.transpose(
                        tp[:], htile[:, 128 * k:128 * (k + 1)], ident[0:4, 0:4])
                    nc.vector.tensor_copy(hT[:, 4 * k:4 * k + 4], tp[:])
                if DBG and t < 3:
                    nc.sync.dma_start(dbg_ht[t], hT[:])
                # ---- scores
                sp = pbig.tile([BL, V], F32, tag="big", name="sp")
                for k in range(8):
                    for n in range(4):
                        wt = stream.tile([128, 512], F32, tag="wout",
                                         name="wout", bufs=3)
                        nc.sync.dma_start(
                            wt[:], woutd[128 * k:128 * (k + 1),
                                         512 * n:512 * (n + 1)])
                        nc.tensor.matmul(
                            sp[:, 512 * n:512 * (n + 1)], hT[:, 4 * k:4 * k + 4],
                            wt[:], start=(k == 0), stop=False)
                for n in range(4):
                    for r in range(2):
                        opst2 = stream.tile([128, 512], F32, tag="opst",
                                            name="opst2", bufs=3)
                        nc.sync.dma_start(
                            opst2[:], op4d[r][:, GD + 512 * n:GD + 512 * (n + 1)])
                        nc.tensor.matmul(
                            sp[:, 512 * n:512 * (n + 1)], probs4[r][:],
                            opst2[:], start=False, stop=(r == 1))
                scb = work.tile([BL, V], F32, tag="big1", name="scb")
                nc.scalar.activation(scb[:], sp[:], AF.Copy)
                nc.sync.dma_start(scores[:, t, :], scb[:])
                if t == T - 1:
                    continue
                # ---- argmax -> tag gather
                shuf = work.tile([128, 64], F32, tag="shuf")
                nc.sync.dma_start(
                    shuf[:],
                    scores[:, t, :].rearrange("b (vg j) -> b vg j", j=64))
                mw = work.tile([128, 8], F32, tag="mw")
                mi = work.tile([128, 8], U32, tag="mi")
                nc.vector.max_with_indices(mw[:], mi[:], shuf[:])
                if DBG and t == 0:
                    nc.sync.dma_start(dbg_shuf[:], shuf[:])
                    nc.sync.dma_start(dbg_mwi[:, 0:8], mw[:])
                    mif32 = work.tile([128, 8], F32, tag="mif32")
                    nc.vector.tensor_copy(mif32[:], mi[:])
                    nc.sync.dma_start(dbg_mwi[:, 8:16], mif32[:])
                two = work.tile([128, 2], F32, tag="two")
                nc.vector.tensor_copy(two[:, 0:1], mw[:, 0:1])
                nc.vector.tensor_copy(two[:, 1:2], mi[:, 0:1])
                tp2 = ppt.tile([2, 128], F32, tag="pt")
                nc.tensor.transpose(tp2[:], two[:], ident[:])
                rowv = work.tile([2, 128], F32, tag="rowv")
                nc.vector.tensor_copy(rowv[:], tp2[:])
                rowi = work.tile([1, 128], F32, tag="rowi")
                nc.sync.dma_start(rowi[:], rowv[1:2, :])
                gl = work.tile([1, 128], F32, tag="gl")
                nc.vector.tensor_add(gl[:], rowi[:], gidxf[:])
                if DBG and t == 0:
                    nc.sync.dma_start(dbg_rowv[:], rowv[:])
                    nc.sync.dma_start(dbg_gl[:], gl[:])
                tagsf = work.tile([BL, 1], F32, tag="tagsf")
                for b in range(4):
                    m8w = work.tile([1, 8], F32, tag="m8w")
                    m8i = work.tile([1, 8], U32, tag="m8i")
                    nc.vector.max_with_indices(
                        m8w[:], m8i[:], rowv[0:1, 32 * b:32 * (b + 1)])
                    ge = work.tile([1, 32], F32, tag="ge")
                    nc.vector.tensor_scalar(
                        ge[:], rowv[0:1, 32 * b:32 * (b + 1)],
                        m8w[:, 0:1], None, op0=ALU.is_ge)
                    tt = work.tile([1, 32], F32, tag="tt")
                    nc.vector.tensor_scalar_add(
                        tt[:], gl[:, 32 * b:32 * (b + 1)], -4096.0)
                    uu = work.tile([1, 32], F32, tag="uu")
                    nc.vector.tensor_mul(uu[:], ge[:], tt[:])
                    sel = work.tile([1, 32], F32, tag="sel")
                    nc.vector.tensor_scalar_add(sel[:], uu[:], 4096.0)
                    tfb = work.tile([1, 1], F32, tag="tfb", bufs=2)
                    nc.vector.tensor_reduce(
                        tfb[:], sel[:], axis=AX.X, op=ALU.min)
                    nc.sync.dma_start(tagsf[b:b + 1, :], tfb[:])
                if DBG and t == 0:
                    nc.sync.dma_start(dbg_tagsf[:], tagsf[:])
                tags_u = work.tile([BL, 1], U32, tag="tagsu")
                nc.vector.tensor_copy(tags_u[:], tagsf[:])
                nc.gpsimd.indirect_dma_start(
                    embX[0:4, :], None, eproj[:],
                    IndirectOffsetOnAxis(ap=tags_u[:], axis=0))
                nc.sync.dma_start(embX[4:8, :], xald[:, t + 1, :])
                # ---- attention for next step
                attention(hT)
    bass_rust.generate_event_semaphores(nc)
    return nc


# ------------------------------------------------------------------ driver
_CACHE = {}


def kernel(**inputs):
    if "nc_enc" not in _CACHE:
        _CACHE["nc_enc"] = build_enc()
        _CACHE["nc_dec"] = build_dec()
    nc_enc, nc_dec = _CACHE["nc_enc"], _CACHE["nc_dec"]
    p = host_prep(inputs)
    emb = np.asarray(inputs["embeddings"], np.float32)  # (32, 64, 512)

    shared_enc = {k: p[k] for k in
                  ["wihT_aug_f", "wihT_aug_b", "whhT_f", "whhT_b", "attn_WT",
                   "w_combT", "walT_aug", "bout_row", "i8", "ident"]}
    in_maps = []
    for c in range(NC_N):
        el = emb[c * BL:(c + 1) * BL]                     # (4, 64, 512)
        embT_aug = np.concatenate(
            [el.reshape(BL * T, D).T,
             np.ones((1, BL * T), np.float32)], axis=0)   # (513, 256)
        m = dict(shared_enc)
        m["embT_aug"] = np.ascontiguousarray(embT_aug)
        in_maps.append(m)
    renc = run_bass_kernel_spmd(nc_enc, in_maps, list(range(NC_N)))

    shared_dec = {k: p[k] for k in
                  ["whhT_dec", "wout_hT", "e_proj", "i8", "ident"]}
    in_maps2 = []
    for c in range(NC_N):
        m = dict(shared_dec)
        r = renc.results[c]
        for k in ["xal", "pti", "op4t", "lastT"]:
            m[k] = r[k]
        in_maps2.append(m)
    rdec = run_bass_kernel_spmd(nc_dec, in_maps2, list(range(NC_N)))

    out = np.concatenate(
        [np.asarray(rdec.results[c]["scores"]) for c in range(NC_N)], axis=0)
    return out.astype(np.float32)                         # (32, 64, 2048)


if __name__ == "__main__":
    z = np.load("/root/problem/ref_cache.npz")
    expected = z["expected"]
    inputs = {k: z[k] for k in z.files if k != "expected"}
    import time
    t0 = time.time()
    actual = kernel(**inputs)
    print("kernel() wall:", time.time() - t0)
    err = np.abs(actual - expected)
    print("max abs err:", err.max(), "scale:", np.abs(expected).max())
    print("rel:", err.max() / np.abs(expected).max())



# revision 10
# speedup vs baseline: 1.6302x; 1.6302x over previous
"""AttentionTagClassifier Trainium2 kernel.

Data-parallel over batch: 32 sequences -> 4 per NeuronCore, weights
replicated.  Two bass programs per core (encoder+precomputes, decoder).

Math (per core, BL=4), validated against reference in numpy first:
 - encoder x-gates as one GEMM with bias folded via an appended ones-row
 - bidirectional LSTM recurrence: stationary = h^T chunks (128,4), the
   weight matrix streams through the PE as the moving operand
 - decoder reformulation:
     * E_proj = tag_embed @ W_emb.T precomputed (host, weight-only), so the
       tag-embedding contribution to the gates is a row gather
     * X_al[t] = aligned_t @ W_al.T + biases precomputed on-chip (GEMM)
     * context never materialized: [W_ctx; Wout_c] @ context ==
       (W_comb @ output_b^T) @ probs_b, with OP_comb precomputed per b and
       the per-step contraction done as a 4-batch block-diagonal matmul
       over T-subchunks straight into the gates/scores PSUM
     * attention scores sc_b = P_b @ h_b with P = output @ attn_W.T
       precomputed; per-b M=1 matmuls with h^T columns as stationary
 - gate order is host-permuted [i f o g] so one sigmoid covers i,f,o
 - argmax over V=2048 via reshape to (128,64) + max_with_indices, then a
   PE transpose and a masked min-reduce for the global (first-max) index
"""
import sys, os
sys.path.insert(0, "/opt/trn_rl_repo")
import numpy as np

import bass_rust
import concourse.bass as bass
import concourse.mybir as mybir
import concourse.tile as tile_mod
from concourse.bass import IndirectOffsetOnAxis
from concourse.bass_utils import run_bass_kernel_spmd
from concourse.bacc import Bacc

F32 = mybir.dt.float32
BF16 = mybir.dt.bfloat16
U32 = mybir.dt.uint32
I32 = mybir.dt.int32
AF = mybir.ActivationFunctionType
ALU = mybir.AluOpType
AX = mybir.AxisListType

B, T, D, H, E, V = 32, 64, 512, 512, 512, 2048
H2 = 2 * H           # 1024
GE = 4 * H           # 2048 encoder gates
GD = 4 * H2          # 4096 decoder gates
NC_N = 8
BL = B // NC_N       # 4
PCOMB = GD + V       # 6144 = [W_ctx(4096); Wout_c(2048)] projection rows


# ---------------------------------------------------------------- tile patch
def _patched_drain_and_barrier(self, tick_clock, wait_clock):
    """This walrus build rejects >1 sync wait per instruction; the Tile tail
    piles every processor's wait onto one Drain.  Split: one Drain each."""
    nc = self.nc
    drain_inst = nc.sync.drain()
    wait_clock.add_sem_waits(
        drain_inst.ins, tile_mod.ScopedClock({None: tick_clock.global_clock})
    )
    si = drain_inst.ins.sync_info
    waits = list(si.on_wait) if si is not None else []
    if len(waits) > 1:
        drain_inst.ins.sync_info = bass_rust.SyncInfo(
            on_wait=[waits[0]], on_update=list(si.on_update)
        )
        for w in waits[1:]:
            d2 = nc.sync.drain()
            d2.ins.sync_info = bass_rust.SyncInfo(on_wait=[w], on_update=[])
    nc.all_engine_barrier()
    assert self.sems is not None
    popped = nc._tile_sem_poison_stack.pop()
    assert popped is self._sem_poison
    nc.clear_and_free_semaphores(list(self.sems.allocated().values()))
    nc.all_engine_barrier()


tile_mod.TileContext._drain_and_barrier = _patched_drain_and_barrier


# ---------------------------------------------------------------- host prep
def host_prep(w):
    p = {}
    for d_ in ("f", "b"):
        wih = np.asarray(w[f"enc_Wih_{d_}"], np.float32)
        whh = np.asarray(w[f"enc_Whh_{d_}"], np.float32)
        bias = np.asarray(w[f"enc_bih_{d_}"], np.float32) + np.asarray(
            w[f"enc_bhh_{d_}"], np.float32)
        # reorder encoder gates [i f g o] -> [i f o g]
        perm = np.concatenate([np.arange(0, H), np.arange(H, 2 * H),
                               np.arange(3 * H, 4 * H), np.arange(2 * H, 3 * H)])
        wih, whh, bias = wih[perm], whh[perm], bias[perm]
        p[f"wihT_aug_{d_}"] = np.ascontiguousarray(
            np.concatenate([wih.T, bias[None, :]], axis=0))       # (513, 2048)
        p[f"whhT_{d_}"] = np.ascontiguousarray(whh.T)             # (512, 2048)

    permd = np.concatenate([np.arange(0, H2), np.arange(H2, 2 * H2),
                            np.arange(3 * H2, 4 * H2), np.arange(2 * H2, 3 * H2)])
    dec_Wih = np.asarray(w["dec_Wih"], np.float32)[permd]
    dec_Whh = np.asarray(w["dec_Whh"], np.float32)[permd]
    dec_bias = (np.asarray(w["dec_bih"], np.float32)
                + np.asarray(w["dec_bhh"], np.float32))[permd]
    W_emb = dec_Wih[:, :E]
    W_ctx = dec_Wih[:, E:E + H2]
    W_al = dec_Wih[:, E + H2:]
    p["whhT_dec"] = np.ascontiguousarray(dec_Whh.T)               # (1024, 4096)
    e_proj = np.asarray(w["tag_embed"], np.float32) @ W_emb.T    # (2048, 4096)
    p["e_proj_lo"] = np.ascontiguousarray(e_proj[:, :GD // 2])
    p["e_proj_hi"] = np.ascontiguousarray(e_proj[:, GD // 2:])
    p["walT_aug"] = np.ascontiguousarray(
        np.concatenate([W_al.T, dec_bias[None, :]], axis=0))      # (1025, 4096)
    Wout = np.asarray(w["Wout"], np.float32)
    p["wout_hT"] = np.ascontiguousarray(Wout[:, :H2].T)           # (1024, 2048)
    p["w_combT"] = np.ascontiguousarray(
        np.concatenate([W_ctx, Wout[:, H2:]], axis=0).T)          # (1024, 6144)
    p["attn_WT"] = np.ascontiguousarray(
        np.asarray(w["attn_W"], np.float32).T)                    # (1024, 1024)
    p["bout_row"] = np.asarray(w["bout"], np.float32)[None, :]    # (1, 2048)
    p["ident"] = np.eye(128, dtype=np.float32)
    p["i8"] = np.concatenate([np.eye(4, dtype=np.float32)] * 2, axis=0)  # (8,4)
    return p


# ------------------------------------------------------------- enc program
def build_enc():
    """v2: strip-tiled recurrence (gate-type col strips), interleaved GEMM
    accumulation chains, 4-way M=32 strips in the OP4T phase."""
    nc = bass.Bass()
    dp = lambda n, s, dt=F32, out=False: nc.declare_dram_parameter(
        n, list(s), dt, isOutput=out)
    embT = dp("embT_aug", (513, BL * T))
    wihT = {d: dp(f"wihT_aug_{d}", (513, GE)) for d in "fb"}
    whhT = {d: dp(f"whhT_{d}", (H, GE)) for d in "fb"}
    attn_WT = dp("attn_WT", (H2, H2))
    w_combT = dp("w_combT", (H2, PCOMB))
    walT = dp("walT_aug", (H2 + 1, GD))
    boutd = dp("bout_row", (1, V))
    i8d = dp("i8", (8, 4))
    identd = dp("ident", (128, 128))

    xal = dp("xal", (BL, T, GD), out=True)
    pti = dp("pti", (8, 128, BL * T), out=True)
    op4t = dp("op4t", (2, 128, PCOMB), out=True)
    lastT_d = dp("lastT", (128, 32), out=True)

    xg = {d: nc.dram_tensor(f"xg_{d}", [BL, T, GE], F32) for d in "fb"}

    with tile_mod.TileContext(nc) as tc:
        with (
            tc.tile_pool(name="res", bufs=1) as res,
            tc.tile_pool(name="wpool", bufs=1) as wpool,
            tc.tile_pool(name="stream", bufs=4) as stream,
            tc.tile_pool(name="work", bufs=3) as work,
            tc.tile_pool(name="pgp", bufs=2, space="PSUM") as pgp,
            tc.tile_pool(name="ptr", bufs=1, space="PSUM") as ptr,
            tc.tile_pool(name="pp3", bufs=3, space="PSUM") as pp3,
        ):
            ident = res.tile([4, 4], F32, tag="ident")
            nc.sync.dma_start(ident[:], identd[0:4, 0:4])
            i8 = res.tile([8, 4], F32, tag="i8")
            nc.sync.dma_start(i8[:], i8d[:])
            ones_row = res.tile([1, BL * T], F32, tag="ones")
            nc.vector.memset(ones_row[:], 1.0)
            boutr = res.tile([1, V], F32, tag="boutr")
            nc.sync.dma_start(boutr[:], boutd[:])

            # ---------------- phase 1: x-gates GEMMs (3-way interleave)
            et = [res.tile([128, BL * T], F32, tag=f"otif{k}", name=f"et{k}")
                  for k in range(4)]
            for k in range(4):
                nc.sync.dma_start(et[k][:], embT[128 * k:128 * (k + 1), :])
            et4 = res.tile([1, BL * T], F32, tag="et4")
            nc.sync.dma_start(et4[:], embT[512:513, :])

            for d_ in "fb":
                wi = [wpool.tile([128, GE], F32, tag=f"bw{k}", name=f"wi{k}")
                      for k in range(4)]
                for k in range(4):
                    nc.sync.dma_start(wi[k][:], wihT[d_][128 * k:128 * (k + 1), :])
                wib = work.tile([1, GE], F32, tag="wib", bufs=1)
                nc.sync.dma_start(wib[:], wihT[d_][512:513, :])
                xgv = xg[d_][:].rearrange("b t g -> (b t) g")
                mns = [(m, n) for m in range(2) for n in range(4)]
                for grp in range(0, 8, 3):
                    sub = mns[grp:grp + 3]
                    pss = {mn: pp3.tile([128, 512], F32, tag="p3",
                                        name=f"ps{mn[0]}{mn[1]}")
                           for mn in sub}
                    for k in range(4):
                        for (m, n) in sub:
                            nc.tensor.matmul(
                                pss[(m, n)][:],
                                et[k][:, 128 * m:128 * (m + 1)],
                                wi[k][:, 512 * n:512 * (n + 1)],
                                start=(k == 0), stop=False)
                    for (m, n) in sub:
                        nc.tensor.matmul(
                            pss[(m, n)][:], et4[:, 128 * m:128 * (m + 1)],
                            wib[:, 512 * n:512 * (n + 1)],
                            start=False, stop=True)
                        sb = work.tile([128, 512], F32, tag="xgout")
                        nc.scalar.activation(sb[:], pss[(m, n)][:], AF.Copy)
                        nc.sync.dma_start(
                            xgv[128 * m:128 * (m + 1), 512 * n:512 * (n + 1)],
                            sb[:])

            # ---------------- phase 2: recurrence (gate-type col strips)
            wr = {d: [wpool.tile([128, GE], F32,
                                 tag=f"bw{(0 if d == chr(102) else 4) + k}",
                                 name=f"whh{d}{k}") for k in range(4)]
                  for d in "fb"}
            for d_ in "fb":
                for k in range(4):
                    nc.sync.dma_start(
                        wr[d_][k][:], whhT[d_][128 * k:128 * (k + 1), :])
            outTI = {d: [res.tile([128, BL * T], F32, tag=f"otix{d}{k}",
                                  name=f"oti{d}{k}") for k in range(4)]
                     for d in "fb"}
            cst = {d: res.tile([BL, H], F32, tag=f"c{d}", name=f"c{d}")
                   for d in "fb"}
            for d_ in "fb":
                nc.vector.memset(cst[d_][:], 0.0)
            lastT = res.tile([128, 32], F32, tag="lastT")

            for step in range(T):
                for d_, t in (("f", step), ("b", T - 1 - step)):
                    tprev = t - 1 if d_ == "f" else t + 1
                    xgt = stream.tile([BL, GE], F32, tag="xgt", bufs=2)
                    nc.sync.dma_start(xgt[:], xg[d_][:, t, :])
                    gp = pgp.tile([128, 512], F32, tag=f"g{d_}", name="gp")
                    if step > 0:
                        for k in range(4):
                            hT_sl = outTI[d_][k][:].rearrange(
                                "p (b t) -> p b t", b=BL)[:, :, tprev]
                            for j in range(4):
                                nc.tensor.matmul(
                                    gp[32 * j:32 * j + 4, :], hT_sl,
                                    wr[d_][k][:, 512 * j:512 * (j + 1)],
                                    start=(k == 0), stop=False,
                                    tile_position=(0, 32 * j))
                    for j in range(4):
                        nc.tensor.matmul(
                            gp[32 * j:32 * j + 4, :], i8[0:4, :],
                            xgt[:, 512 * j:512 * (j + 1)],
                            start=(step == 0), stop=True,
                            tile_position=(0, 32 * j))
                    # cell ([i f o g] strips at partitions 0/32/64/96)
                    sfo = work.tile([BL, 3 * H], F32, tag=f"sfo{d_}", bufs=1,
                                    name=f"sfo{d_}")
                    for jj in range(3):
                        nc.scalar.activation(
                            sfo[:, H * jj:H * (jj + 1)],
                            gp[32 * jj:32 * jj + 4, :], AF.Sigmoid)
                    tg = work.tile([BL, H], F32, tag=f"tg{d_}", bufs=1,
                                   name=f"tg{d_}")
                    nc.scalar.activation(tg[:], gp[96:100, :], AF.Tanh)
                    nc.vector.tensor_mul(cst[d_][:], cst[d_][:], sfo[:, H:2 * H])
                    nc.vector.tensor_mul(tg[:], tg[:], sfo[:, 0:H])
                    nc.vector.tensor_add(cst[d_][:], cst[d_][:], tg[:])
                    nc.scalar.activation(tg[:], cst[d_][:], AF.Tanh)
                    htile = work.tile([BL, H], F32, tag=f"h{d_}", bufs=1,
                                      name=f"h{d_}")
                    nc.vector.tensor_mul(htile[:], sfo[:, 2 * H:3 * H], tg[:])
                    for k in range(4):
                        tp = ptr.tile([128, BL], F32, tag="tr")
                        nc.tensor.transpose(
                            tp[:], htile[:, 128 * k:128 * (k + 1)],
                            ident[:])
                        dst = outTI[d_][k][:].rearrange(
                            "p (b t) -> p b t", b=BL)[:, :, t]
                        nc.vector.tensor_copy(dst, tp[:])
                    if (d_ == "f" and step == T - 1) or (d_ == "b" and step == 0):
                        off = 0 if d_ == "f" else 16
                        for k in range(4):
                            src = outTI[d_][k][:].rearrange(
                                "p (b t) -> p b t", b=BL)[:, :, T - 1]
                            nc.vector.tensor_copy(
                                lastT[:, off + 4 * k:off + 4 * k + 4], src)
            nc.sync.dma_start(lastT_d[:], lastT[:])

            oti8 = outTI["f"] + outTI["b"]  # d-chunks 0..7 of [out_f; out_b]

            # ---------------- phase 3a: X_al (3-way interleave)
            xalv = xal[:].rearrange("b t g -> (b t) g")
            wbias = res.tile([1, GD], F32, tag="walbias")
            nc.sync.dma_start(wbias[:], walT[H2:H2 + 1, :])
            mns = [(m, n) for m in range(2) for n in range(8)]
            for grp in range(0, 16, 3):
                sub = mns[grp:grp + 3]
                pss = {mn: pp3.tile([128, 512], F32, tag="p3",
                                    name=f"pa{mn[0]}{mn[1]}") for mn in sub}
                for k in range(8):
                    for (m, n) in sub:
                        wals = stream.tile([128, 512], F32, tag="wals",
                                           name="wals", bufs=4)
                        nc.sync.dma_start(
                            wals[:], walT[128 * k:128 * (k + 1),
                                          512 * n:512 * (n + 1)])
                        nc.tensor.matmul(
                            pss[(m, n)][:], oti8[k][:, 128 * m:128 * (m + 1)],
                            wals[:], start=(k == 0), stop=False)
                for (m, n) in sub:
                    nc.tensor.matmul(
                        pss[(m, n)][:], ones_row[:, 128 * m:128 * (m + 1)],
                        wbias[:, 512 * n:512 * (n + 1)], start=False, stop=True)
                    sb = work.tile([128, 512], F32, tag="xgout", name="xalout")
                    nc.scalar.activation(sb[:], pss[(m, n)][:], AF.Copy)
                    nc.sync.dma_start(
                        xalv[128 * m:128 * (m + 1), 512 * n:512 * (n + 1)],
                        sb[:])

            # ---------------- phase 3b: PTI (3-way interleave)
            for grp in range(0, 8, 3):
                sub = list(range(grp, min(grp + 3, 8)))
                pss = {e: pp3.tile([128, BL * T], F32, tag="p3",
                                   name=f"pe{e}") for e in sub}
                for k in range(8):
                    for e in sub:
                        awt = stream.tile([128, 128], F32, tag="awt", bufs=4)
                        nc.sync.dma_start(
                            awt[:], attn_WT[128 * k:128 * (k + 1),
                                            128 * e:128 * (e + 1)])
                        nc.tensor.matmul(pss[e][:], awt[:], oti8[k][:],
                                         start=(k == 0), stop=(k == 7))
                for e in sub:
                    sb = work.tile([128, BL * T], F32, tag="ptiout")
                    nc.scalar.activation(sb[:], pss[e][:], AF.Copy)
                    nc.sync.dma_start(pti[e], sb[:])

            # ------------- phase 3c: OP4T (M=32, 4-way col strips, 2 tiles)
            for n in range(PCOMB // 512):
                ppz = [pp3.tile([128, 512], F32, tag="p3", name=f"pp{z}")
                       for z in range(2)]
                is_v = n >= GD // 512
                for k in range(8):
                    wcb = stream.tile([128, 512], F32, tag="wcb", bufs=4)
                    nc.sync.dma_start(
                        wcb[:], w_combT[128 * k:128 * (k + 1),
                                        512 * n:512 * (n + 1)])
                    for b in range(4):
                        for r in range(2):
                            s = 32 * (2 * (b % 2) + r)
                            nc.tensor.matmul(
                                ppz[b // 2][s:s + 32, :],
                                oti8[k][:, 64 * b + 32 * r:][:, 0:32],
                                wcb[:], start=(k == 0),
                                stop=(k == 7 and not is_v),
                                tile_position=(0, s))
                if is_v:
                    bsl = boutr[:, 512 * n - GD:512 * (n + 1) - GD]
                    for b in range(4):
                        for r in range(2):
                            s = 32 * (2 * (b % 2) + r)
                            nc.tensor.matmul(
                                ppz[b // 2][s:s + 32, :],
                                ones_row[:, 0:32], bsl,
                                start=False, stop=True,
                                tile_position=(0, s))
                for b in range(4):
                    for r in range(2):
                        s = 32 * (2 * (b % 2) + r)
                        opc = work.tile([32, 512], F32, tag="op4c",
                                        name="op4c")
                        nc.vector.tensor_copy(opc[:], ppz[b // 2][s:s + 32, :])
                        nc.sync.dma_start(
                            op4t[r][32 * b:32 * (b + 1),
                                    512 * n:512 * (n + 1)],
                            opc[:])
    bass_rust.generate_event_semaphores(nc)
    return nc


# ------------------------------------------------------------- dec program
def build_dec():
    """v2: strip-tiled (tile_position col groups) decoder.

    Gate PSUM layout: two banks pgA/pgB ([128,512] each); strip j (gate
    type j in host order [i f o g]) lives at partitions 32j..32j+4; bank
    A holds hidden 0:512, bank B hidden 512:1024 of each gate type.
    Scores PSUM: one bank, strip j = V cols 512j..512j+512.
    whh + op4t-gates resident in SBUF; wout + op4t-scores streamed.
    Argmax via max/max_index top-8 (first-occurrence == np.argmax since
    top-1 is unique at f32 precision).  Softmax exp via tanh identity to
    avoid ACT table swaps: e^x = (1+t)/(1-t), t = tanh(x/2).
    """
    nc = bass.Bass()
    dp = lambda n, s, dt=F32, out=False: nc.declare_dram_parameter(
        n, list(s), dt, isOutput=out)
    whhd = dp("whhT_dec", (H2, GD))
    woutd = dp("wout_hT", (H2, V))
    eproj = [dp("e_proj_lo", (V, GD // 2)), dp("e_proj_hi", (V, GD // 2))]
    xald = dp("xal", (BL, T, GD))
    ptid = dp("pti", (8, 128, BL * T))
    op4d = dp("op4t", (2, 128, PCOMB))
    lastd = dp("lastT", (128, 32))
    i8d = dp("i8", (8, 4))
    identd = dp("ident", (128, 128))
    scores = dp("scores", (BL, T, V), out=True)

    with tile_mod.TileContext(nc) as tc:
        with (
            tc.tile_pool(name="res", bufs=1) as res,
            tc.tile_pool(name="stream", bufs=1) as stream,
            tc.tile_pool(name="work", bufs=1) as work,
            tc.tile_pool(name="pg", bufs=2, space="PSUM") as pgp,
            tc.tile_pool(name="psc", bufs=2, space="PSUM") as pscp,
            tc.tile_pool(name="ptr", bufs=1, space="PSUM") as ptrp,
            tc.tile_pool(name="pat", bufs=1, space="PSUM") as patp,
        ):
            ident = res.tile([4, 4], F32, tag="ident")
            nc.sync.dma_start(ident[:], identd[0:4, 0:4])
            i8 = res.tile([8, 4], F32, tag="i8")
            nc.sync.dma_start(i8[:], i8d[:])
            w = [res.tile([128, GD], F32, tag=f"w{k}", name=f"w{k}")
                 for k in range(8)]
            for k in range(8):
                nc.sync.dma_start(w[k][:], whhd[128 * k:128 * (k + 1), :])
            p = [res.tile([128, BL * T], F32, tag=f"p{k}", name=f"p{k}")
                 for k in range(8)]
            for k in range(8):
                nc.sync.dma_start(p[k][:], ptid[k])
            lastT = res.tile([128, 32], F32, tag="lastT")
            nc.sync.dma_start(lastT[:], lastd[:])
            probs4 = [res.tile([128, 4], F32, tag=f"pr4{r}", name=f"pr4{r}")
                      for r in range(2)]
            for r in range(2):
                nc.vector.memset(probs4[r][:], 0.0)
            hT = res.tile([128, 32], F32, tag="hT")
            cs = [res.tile([BL, H], F32, tag=f"c{x}", name=f"cs{x}")
                  for x in range(2)]
            for x in range(2):
                nc.vector.memset(cs[x][:], 0.0)
            embXc = res.tile([8, GD // 2], F32, tag="embXc")
            nc.vector.memset(embXc[0:4, :], 0.0)
            onesP = res.tile([97, 1], F32, tag="onesP")
            nc.vector.memset(onesP[:], 1.0)

            def attention(hT_src):
                """probs for the next step, batched over b via col strips."""
                scp = patp.tile([128, T], F32, tag="at", name="scp")
                for k in range(8):
                    for b in range(4):
                        nc.tensor.matmul(
                            scp[32 * b:32 * b + 1, :],
                            hT_src[:, 4 * k + b:4 * k + b + 1],
                            p[k][:, T * b:T * (b + 1)],
                            start=(k == 0), stop=(k == 7),
                            tile_position=(0, 32 * b))
                # e^sc = (1+t)/(1-t), t = tanh(sc/2); rows {0,32,64,96}
                th = work.tile([97, T], F32, tag="ath", bufs=1)
                nc.scalar.activation(th[:], scp[0:97, :], AF.Tanh, scale=0.5)
                num = work.tile([97, T], F32, tag="anum", bufs=1)
                nc.vector.tensor_scalar_add(num[:], th[:], 1.0)
                den = work.tile([97, T], F32, tag="aden", bufs=1)
                nc.vector.tensor_scalar(
                    den[:], th[:], -1.0, 1.0, op0=ALU.mult, op1=ALU.add)
                nc.vector.reciprocal(den[:], den[:])
                nc.vector.tensor_mul(num[:], num[:], den[:])
                ssum = work.tile([97, 1], F32, tag="assum", bufs=1)
                nc.vector.tensor_reduce(ssum[:], num[:], axis=AX.X, op=ALU.add)
                nc.vector.reciprocal(ssum[:], ssum[:])
                nc.vector.tensor_scalar_mul(num[:], num[:], ssum[:])
                for b in range(4):
                    pT = ptrp.tile([T, 1], F32, tag="tr", name="pT")
                    nc.tensor.transpose(
                        pT[:], num[32 * b:32 * b + 1, :],
                        onesP[32 * b:32 * b + 1, 0:1],
                        tile_position=(32 * b, 0))
                    for r in range(2):
                        nc.vector.tensor_copy(
                            probs4[r][32 * b:32 * b + 32, b:b + 1],
                            pT[32 * r:32 * r + 32, 0:1])

            attention(lastT)

            mws = [None, None]
            for t in range(T):
                # ---- gates: whh (resident) + ctx (resident og) ----------
                pg = [pgp.tile([128, 512], F32, tag=f"g{h}", name=f"pg{h}")
                      for h in range(2)]
                if t > 0:
                    for k in range(8):
                        for h in range(2):
                            for j in range(4):
                                nc.tensor.matmul(
                                    pg[h][32 * j:32 * j + 4, :],
                                    hT[:, 4 * k:4 * k + 4],
                                    w[k][:, 1024 * j + 512 * h:][:, 0:512],
                                    start=(k == 0), stop=False,
                                    tile_position=(0, 32 * j))
                for r in range(2):
                    for h in range(2):
                        for j in range(4):
                            ogt = stream.tile([128, 512], F32, tag="ogst",
                                              name="ogt", bufs=5)
                            nc.sync.dma_start(
                                ogt[:],
                                op4d[r][:, 1024 * j + 512 * h:][:, 0:512])
                            nc.tensor.matmul(
                                pg[h][32 * j:32 * j + 4, :],
                                probs4[r][:], ogt[:],
                                start=(r == 0 and t == 0), stop=False,
                                tile_position=(0, 32 * j))
                # ---- embX: e_proj[tag] + xal row, in two column rounds --
                for m in range(2):
                    if t > 0:
                        nc.gpsimd.indirect_dma_start(
                            embXc[0:4, :], None, eproj[m][:],
                            IndirectOffsetOnAxis(
                                ap=mws[1][:, 0:1], axis=0))
                    nc.sync.dma_start(
                        embXc[4:8, :], xald[:, t, 2048 * m:2048 * (m + 1)])
                    for j, h in (((2 * m), 0), ((2 * m), 1),
                                 ((2 * m + 1), 0), ((2 * m + 1), 1)):
                        col = (1024 * j + 512 * h) % 2048
                        nc.tensor.matmul(
                            pg[h][32 * j:32 * j + 4, :], i8[:],
                            embXc[:, col:col + 512],
                            start=False, stop=True,
                            tile_position=(0, 32 * j))
                # ---- activations + cell, per hidden-half ----------------
                hs = []
                for h in range(2):
                    sfo = work.tile([4, 1536], F32, tag=f"sfo{h}", bufs=1,
                                    name=f"sfo{h}")
                    for jj in range(3):
                        nc.scalar.activation(
                            sfo[:, 512 * jj:512 * (jj + 1)],
                            pg[h][32 * jj:32 * jj + 4, :], AF.Sigmoid)
                    tg = work.tile([4, 512], F32, tag=f"tg{h}", bufs=1,
                                   name=f"tg{h}")
                    nc.scalar.activation(tg[:], pg[h][96:100, :], AF.Tanh)
                    nc.vector.tensor_mul(cs[h][:], cs[h][:], sfo[:, 512:1024])
                    nc.vector.tensor_mul(tg[:], tg[:], sfo[:, 0:512])
                    nc.vector.tensor_add(cs[h][:], cs[h][:], tg[:])
                    nc.scalar.activation(tg[:], cs[h][:], AF.Tanh)
                    ht = work.tile([4, 512], F32, tag=f"h{h}", bufs=1,
                                   name=f"ht{h}")
                    nc.vector.tensor_mul(ht[:], sfo[:, 1024:1536], tg[:])
                    hs.append(ht)
                for k in range(8):
                    tp = ptrp.tile([128, BL], F32, tag="tr", name="tp")
                    nc.tensor.transpose(
                        tp[:], hs[k // 4][:, 128 * (k % 4):][:, 0:128],
                        ident[:])
                    nc.vector.tensor_copy(hT[:, 4 * k:4 * k + 4], tp[:])
                # ---- scores: streamed wout + streamed op4t-scores -------
                psc = pscp.tile([128, 512], F32, tag="sc", name="psc")
                for k in range(8):
                    for j in range(4):
                        wt = stream.tile([128, 512], F32, tag="wout",
                                         name="wt", bufs=6)
                        nc.sync.dma_start(
                            wt[:], woutd[128 * k:128 * (k + 1),
                                         512 * j:512 * (j + 1)])
                        nc.tensor.matmul(
                            psc[32 * j:32 * j + 4, :], hT[:, 4 * k:4 * k + 4],
                            wt[:], start=(k == 0), stop=False,
                            tile_position=(0, 32 * j))
                for r in range(2):
                    for j in range(4):
                        ot = stream.tile([128, 512], F32, tag="opsc",
                                         name="ot", bufs=4)
                        nc.sync.dma_start(
                            ot[:], op4d[r][:, GD + 512 * j:GD + 512 * (j + 1)])
                        nc.tensor.matmul(
                            psc[32 * j:32 * j + 4, :], probs4[r][:], ot[:],
                            start=False, stop=(r == 1),
                            tile_position=(0, 32 * j))
                scb = work.tile([4, V], F32, tag="scb", bufs=1)
                for j in range(4):
                    nc.vector.tensor_copy(
                        scb[:, 512 * j:512 * (j + 1)],
                        psc[32 * j:32 * j + 4, :])
                nc.sync.dma_start(scores[:, t, :], scb[:])
                if t == T - 1:
                    continue
                # ---- argmax (top-1 of top-8) ----------------------------
                mw = work.tile([4, 8], F32, tag="mw", bufs=2)
                mi = work.tile([4, 8], U32, tag="mi", bufs=2)
                nc.vector.max(mw[:], scb[:])
                nc.vector.max_index(mi[:], mw[:], scb[:])
                mws = [mws[1], mi]
                # ---- attention for next step ----------------------------
                attention(hT)
    bass_rust.generate_event_semaphores(nc)
    return nc


# ------------------------------------------------------------------ driver
_CACHE = {}


def kernel(**inputs):
    if "nc_enc" not in _CACHE:
        _CACHE["nc_enc"] = build_enc()
        _CACHE["nc_dec"] = build_dec()
    nc_enc, nc_dec = _CACHE["nc_enc"], _CACHE["nc_dec"]
    p = host_prep(inputs)
    emb = np.asarray(inputs["embeddings"], np.float32)  # (32, 64, 512)

    shared_enc = {k: p[k] for k in
                  ["wihT_aug_f", "wihT_aug_b", "whhT_f", "whhT_b", "attn_WT",
                   "w_combT", "walT_aug", "bout_row", "i8", "ident"]}
    in_maps = []
    for c in range(NC_N):
        el = emb[c * BL:(c + 1) * BL]                     # (4, 64, 512)
        embT_aug = np.concatenate(
            [el.reshape(BL * T, D).T,
             np.ones((1, BL * T), np.float32)], axis=0)   # (513, 256)
        m = dict(shared_enc)
        m["embT_aug"] = np.ascontiguousarray(embT_aug)
        in_maps.append(m)
    renc = run_bass_kernel_spmd(nc_enc, in_maps, list(range(NC_N)))

    shared_dec = {k: p[k] for k in
                  ["whhT_dec", "wout_hT", "e_proj_lo", "e_proj_hi",
                   "i8", "ident"]}
    in_maps2 = []
    for c in range(NC_N):
        m = dict(shared_dec)
        r = renc.results[c]
        for k in ["xal", "pti", "op4t", "lastT"]:
            m[k] = r[k]
        in_maps2.append(m)
    rdec = run_bass_kernel_spmd(nc_dec, in_maps2, list(range(NC_N)))

    out = np.concatenate(
        [np.asarray(rdec.results[c]["scores"]) for c in range(NC_N)], axis=0)
    return out.astype(np.float32)                         # (32, 64, 2048)


if __name__ == "__main__":
    z = np.load("/root/problem/ref_cache.npz")
    expected = z["expected"]
    inputs = {k: z[k] for k in z.files if k != "expected"}
    import time
    t0 = time.time()
    actual = kernel(**inputs)
    print("kernel() wall:", time.time() - t0)
    err = np.abs(actual - expected)
    print("max abs err:", err.max(), "scale:", np.abs(expected).max())
    print("rel:", err.max() / np.abs(expected).max())

